# revision 1
# baseline (speedup 1.0000x reference)
"""FNet transformer block kernel for Trainium2 (8 NeuronCores, data-parallel over batch).

Math notes
----------
reference computes, per batch b:
    ft  = Re( FFT_seq( FFT_hid( FFT_hid( x ))))        (hidden FFT applied twice)
    u   = x + ft;  t = LayerNorm(u) * g + beta
    out = (gelu(t @ w1 + b1) @ w2 + b2) * mask

Double FFT along hidden (D=1024):  (F_D^2 x)[d] = D * x[(-d) mod D]  (real).
So with w[t, d] = 1024 * x[t, (-d) mod 1024]:
    ft = Re(F_S) @ w = C @ w,   C[s, t] = cos(2*pi*s*t/2048)   (S=2048)
C is symmetric in both index reflections: C[s,t] = C[2048-s,t] = C[s,2048-t].
Folding both halves turns the 2048x2048 cosine transform into a ~1025x1025 one:
    Z.T = wf.T @ Cf,  wf = t-folded w (1025 rows, padded to 1152),
    Cf[t,s] = C[t,s] for t,s in [0,1024] (padded to 1152x1056),
    ft.T[:, s] = Z.T[:, s] for s<=1024, else Z.T[:, 2048-s]  (free-dim mirror).

Everything downstream runs with activations transposed (d on partitions) until
FFN1, whose stationary operand is u.T, which flips the result back to natural
token-major layout; FFN2 flips again via PE transposes of H.

LayerNorm is applied through the FFN1 matmul:
    P[s,j] = r[s]*(A[s,j] - m[s]*wsum1[j]) + b1p[j],  A = u.T.T @ w1p
using two rank-1 (K=1) matmul updates into the PSUM accumulator and a
per-partition ACT scale r[s] fused into the GELU activation.
gamma/beta are folded into w1p/b1p on the host.
"""

import sys
from contextlib import ExitStack

import numpy as np

sys.path.insert(0, "/opt/trn_rl_repo")

import concourse.bass as bass  # noqa: E402
import concourse.mybir as mybir  # noqa: E402
import concourse.tile as tile
from concourse import bacc  # noqa: E402
from concourse.bass_utils import run_bass_kernel_spmd  # noqa: E402
from concourse.masks import make_identity  # noqa: E402

S, D = 2048, 1024
TF = 1152  # folded-t rows: 1025 padded up to 9*128
SF = 1056  # folded-s cols: 1025 padded up to 1056
NCORES = 8
LN_EPS = 1e-5
EPS_P = float(D) * float(D) * LN_EPS
F32 = mybir.dt.float32
F32R = mybir.dt.float32r
KT = TF // 128  # 9
DT = D // 128   # 8
BW = 256        # s-block width for the streaming phase
NB = S // BW    # 8
NCHUNKS = [(0, 512), (512, 512), (1024, 32)]  # FFT output column chunks of SF


def _r(ap):
    return ap.bitcast(F32R)


def _emit_kernel(ctx: ExitStack, tc: tile.TileContext, xT, wf, cf, w1p, w2,
                 wsum1r, b1pdr, b2r, onescol, onesrow, out):
    nc = tc.nc
    f32 = F32

    cpool = ctx.enter_context(tc.tile_pool(name="consts", bufs=1))
    ones_col = cpool.tile([128, 1], F32R, tag="ones_col")
    nc.sync.dma_start(ones_col[:], onescol[:])
    ones_row = cpool.tile([1, 128], F32R, tag="ones_row")
    nc.sync.dma_start(ones_row[:], onesrow[:])
    ones_11 = ones_row[0:1, 0:1]
    ident = cpool.tile([128, 128], f32, tag="ident")
    make_identity(nc, ident[:])
    eps_t = cpool.tile([1, 1], f32, tag="eps_t")
    nc.gpsimd.memset(eps_t[:], EPS_P)
    zero_col = cpool.tile([128, 1], f32, tag="zero_col")
    nc.gpsimd.memset(zero_col[:], 0.0)
    wsum1_s = cpool.tile([1, D], F32R, tag="wsum1")
    nc.sync.dma_start(wsum1_s[:], wsum1r[:])
    b1pd_s = cpool.tile([1, D], F32R, tag="b1pd")
    nc.sync.dma_start(b1pd_s[:], b1pdr[:])
    b2_s = cpool.tile([1, D], F32R, tag="b2")
    nc.sync.dma_start(b2_s[:], b2r[:])

    # w1 stays resident through the whole kernel
    w1pool = ctx.enter_context(tc.tile_pool(name="w1", bufs=1))
    w1_s = []
    for dt_ in range(DT):
        t_ = w1pool.tile([128, D], F32R, tag=f"w1_{dt_}")
        nc.sync.dma_start(t_[:], w1p[dt_ * 128:(dt_ + 1) * 128, :])
        w1_s.append(t_)

    # Z.T (folded FFT output), resident
    zpool = ctx.enter_context(tc.tile_pool(name="zt", bufs=1))
    zt_s = [zpool.tile([128, SF], f32, tag=f"zt{m}", name=f"zt{m}")
            for m in range(DT)]

    # ---------------- Phase 1: folded cosine transform ----------------
    with tc.tile_pool(name="fft_in", bufs=1) as fpool, \
         tc.tile_pool(name="fft_ps", bufs=4, space="PSUM") as fps:
        wf_s, cf_s = [], []
        for kt in range(KT):
            a = fpool.tile([128, D], F32R, tag=f"wf{kt}")
            nc.sync.dma_start(a[:], wf[kt * 128:(kt + 1) * 128, :])
            wf_s.append(a)
            c = fpool.tile([128, SF], F32R, tag=f"cf{kt}")
            nc.sync.dma_start(c[:], cf[kt * 128:(kt + 1) * 128, :])
            cf_s.append(c)
        for mt in range(DT):
            for (n0, nw) in NCHUNKS:
                ps = fps.tile([128, 512], f32, tag="fftps")
                for kt in range(KT):
                    nc.tensor.matmul(
                        ps[:, :nw],
                        _r(wf_s[kt][:, mt * 128:(mt + 1) * 128]),
                        _r(cf_s[kt][:, n0:n0 + nw]),
                        start=(kt == 0), stop=(kt == KT - 1),
                    )
                nc.vector.tensor_copy(zt_s[mt][:, n0:n0 + nw], ps[:, :nw])

    # w2 resident (allocated after the FFT pools release their SBUF)
    w2pool = ctx.enter_context(tc.tile_pool(name="w2", bufs=1))
    w2_s = []
    for jt in range(DT):
        t_ = w2pool.tile([128, D], F32R, tag=f"w2_{jt}")
        nc.sync.dma_start(t_[:], w2[jt * 128:(jt + 1) * 128, :])
        w2_s.append(t_)

    # ---------------- Phase 2: streamed residual+LN+FFN ----------------
    xpool = ctx.enter_context(tc.tile_pool(name="xt", bufs=9))
    upool = ctx.enter_context(tc.tile_pool(name="u", bufs=9))
    usqpool = ctx.enter_context(tc.tile_pool(name="usq", bufs=9))
    stpool = ctx.enter_context(tc.tile_pool(name="strow", bufs=2))
    hpool = ctx.enter_context(tc.tile_pool(name="h", bufs=2))
    htpool = ctx.enter_context(tc.tile_pool(name="ht", bufs=2))
    opool = ctx.enter_context(tc.tile_pool(name="o", bufs=2))
    ffps = ctx.enter_context(tc.tile_pool(name="ffps", bufs=1, space="PSUM"))
    ops_ = ctx.enter_context(tc.tile_pool(name="ops", bufs=1, space="PSUM"))
    htps = ctx.enter_context(tc.tile_pool(name="htps", bufs=2, space="PSUM"))
    stps = ctx.enter_context(tc.tile_pool(name="stps", bufs=1, space="PSUM"))
    svps = ctx.enter_context(tc.tile_pool(name="svps", bufs=1, space="PSUM"))

    for blk in range(NB):
        s0 = blk * BW
        # u.T block = x.T block + mirrored Z.T columns
        u_t = []
        xt_ts = []
        for dt_ in range(DT):
            xt_t = xpool.tile([128, BW], f32, tag="xt")
            xt_ts.append(xt_t)
            nc.sync.dma_start(xt_t[:], xT[dt_ * 128:(dt_ + 1) * 128, s0:s0 + BW])
            u = upool.tile([128, BW], F32R, tag="u")
            z = zt_s[dt_]
            if s0 + BW <= 1025:
                nc.vector.tensor_add(u[:], xt_t[:], z[:, s0:s0 + BW])
            elif s0 == 1024:
                nc.vector.tensor_add(u[:, 0:1], xt_t[:, 0:1], z[:, 1024:1025])
                nc.vector.tensor_add(u[:, 1:BW], xt_t[:, 1:BW],
                                     z[:, 1023:1024 - BW:-1])
            else:
                nc.vector.tensor_add(u[:], xt_t[:],
                                     z[:, 2048 - s0:2048 - s0 - BW:-1])
            u_t.append(u)

        # token stats via ones-matmuls: S1 and S2 side by side on partition 0
        # (matmul dst must start at partition 0)
        s12 = stps.tile([1, 2 * BW], f32, tag="s12")
        for dt_ in range(DT):
            nc.tensor.matmul(s12[0:1, 0:BW], _r(ones_col[:]), _r(u_t[dt_][:]),
                             start=(dt_ == 0), stop=(dt_ == DT - 1))
        usq_t = []
        for dt_ in range(DT):
            usq = usqpool.tile([128, BW], F32R, tag="usq")
            nc.vector.tensor_mul(usq[:], u_t[dt_][:], u_t[dt_][:])
            usq_t.append(usq)
        for dt_ in range(DT):
            nc.tensor.matmul(s12[0:1, BW:2 * BW], _r(ones_col[:]),
                             _r(usq_t[dt_][:]),
                             start=(dt_ == 0), stop=(dt_ == DT - 1))

        s1r = stpool.tile([1, BW], f32, tag="s1r")
        nc.vector.tensor_copy(s1r[:], s12[0:1, 0:BW])
        s2r = stpool.tile([1, BW], f32, tag="s2r")
        nc.vector.tensor_copy(s2r[:], s12[0:1, BW:2 * BW])
        sqr = stpool.tile([1, BW], f32, tag="sqr")
        nc.vector.tensor_mul(sqr[:], s1r[:], s1r[:])
        vr = stpool.tile([1, BW], f32, tag="vr")
        # vr = 1024*S2 - S1^2  (= 1024^2 * var)
        nc.vector.scalar_tensor_tensor(
            out=vr[:], in0=s2r[:], scalar=float(D), in1=sqr[:],
            op0=mybir.AluOpType.mult, op1=mybir.AluOpType.subtract)
        svr = stpool.tile([1, BW], F32R, tag="svr")
        # svr = sqrt(vr + 1024^2*eps) = 1024*sqrt(var+eps)
        nc.scalar.activation(svr[:], vr[:], mybir.ActivationFunctionType.Sqrt,
                             bias=eps_t[0:1, 0:1], scale=1.0)
        mnegr = stpool.tile([1, BW], F32R, tag="mnegr")
        nc.vector.tensor_scalar_mul(mnegr[:], s1r[:], -1.0 / float(D))

        for i in range(BW // 128):
            st = blk * (BW // 128) + i
            isl = slice(i * 128, (i + 1) * 128)
            # r column: reciprocal of sv, times 1024. Plain fp32 matmul —
            # N=1 violates fp32r ISA restrictions (and is cost-trivial).
            svc = svps.tile([128, 1], f32, tag="svc")
            nc.tensor.matmul(svc[:], svr[0:1, isl].bitcast(F32),
                             ones_11.bitcast(F32), start=True, stop=True)
            rcol = stpool.tile([128, 1], f32, tag="rcol")
            nc.vector.reciprocal(rcol[:], svc[:])
            rcol2 = stpool.tile([128, 1], f32, tag="rcol2")
            nc.vector.tensor_scalar_mul(rcol2[:], rcol[:], float(D))

            # FFN1: A = u.T.T @ w1p  (+ rank-1 LN corrections)
            psA = ffps.tile([128, D], f32, tag="psA")
            for dt_ in range(DT):
                for (c0, cw) in [(0, 512), (512, 512)]:
                    nc.tensor.matmul(psA[:, c0:c0 + cw],
                                     _r(u_t[dt_][:, isl]),
                                     _r(w1_s[dt_][:, c0:c0 + cw]),
                                     start=(dt_ == 0), stop=False)
            for (c0, cw) in [(0, 512), (512, 512)]:
                nc.tensor.matmul(psA[:, c0:c0 + cw], _r(mnegr[0:1, isl]),
                                 _r(wsum1_s[0:1, c0:c0 + cw]),
                                 start=False, stop=False)
                nc.tensor.matmul(psA[:, c0:c0 + cw], _r(svr[0:1, isl]),
                                 _r(b1pd_s[0:1, c0:c0 + cw]),
                                 start=False, stop=True)
            h_t = hpool.tile([128, D], f32, tag="h")
            nc.scalar.activation(h_t[:], psA[:],
                                 mybir.ActivationFunctionType.Gelu,
                                 bias=zero_col[:, 0:1], scale=rcol2[:, 0:1])

            # transpose H for FFN2
            htsb = htpool.tile([128, D], F32R, tag="htsb")
            for half in range(2):
                psHT = htps.tile([128, 512], f32, tag="psHT")
                for q in range(4):
                    jt = half * 4 + q
                    nc.tensor.transpose(psHT[:, q * 128:(q + 1) * 128],
                                        h_t[:, jt * 128:(jt + 1) * 128],
                                        ident[:])
                nc.vector.tensor_copy(htsb[:, half * 512:(half + 1) * 512],
                                      psHT[:])

            # FFN2: out = H @ w2 + b2
            psO = ops_.tile([128, D], f32, tag="psO")
            for jt in range(DT):
                for (c0, cw) in [(0, 512), (512, 512)]:
                    nc.tensor.matmul(psO[:, c0:c0 + cw],
                                     _r(htsb[:, jt * 128:(jt + 1) * 128]),
                                     _r(w2_s[jt][:, c0:c0 + cw]),
                                     start=(jt == 0), stop=False)
            for (c0, cw) in [(0, 512), (512, 512)]:
                nc.tensor.matmul(psO[:, c0:c0 + cw], _r(ones_row[:]),
                                 _r(b2_s[0:1, c0:c0 + cw]),
                                 start=False, stop=True)
            o_t = opool.tile([128, D], f32, tag="o")
            nc.scalar.copy(o_t[:], psO[:])
            nc.sync.dma_start(out[st * 128:(st + 1) * 128, :], o_t[:])


_NC_CACHE = {}


def _build_nc():
    if "nc" in _NC_CACHE:
        return _NC_CACHE["nc"]
    nc = bacc.Bacc("TRN2", target_bir_lowering=False, debug=False)
    xT = nc.declare_dram_parameter("xT", [D, S], F32, isOutput=False)
    wf = nc.declare_dram_parameter("wf", [TF, D], F32R, isOutput=False)
    cf = nc.declare_dram_parameter("cf", [TF, SF], F32R, isOutput=False)
    w1p = nc.declare_dram_parameter("w1p", [D, D], F32R, isOutput=False)
    w2 = nc.declare_dram_parameter("w2", [D, D], F32R, isOutput=False)
    wsum1r = nc.declare_dram_parameter("wsum1r", [1, D], F32R, isOutput=False)
    b1pdr = nc.declare_dram_parameter("b1pdr", [1, D], F32R, isOutput=False)
    b2r = nc.declare_dram_parameter("b2r", [1, D], F32R, isOutput=False)
    onescol = nc.declare_dram_parameter("onescol", [128, 1], F32R, isOutput=False)
    onesrow = nc.declare_dram_parameter("onesrow", [1, 128], F32R, isOutput=False)
    out = nc.declare_dram_parameter("out", [S, D], F32, isOutput=True)
    with tile.TileContext(nc) as tc:
        with ExitStack() as ctx:
            _emit_kernel(ctx, tc, xT, wf, cf, w1p, w2, wsum1r, b1pdr, b2r,
                         onescol, onesrow, out)
    nc.compile()
    _NC_CACHE["nc"] = nc
    return nc


def _host_prep(x, ln_g, ln_b, w1, b1, w2, b2):
    """Build per-core and shared device inputs (all float32)."""
    B = x.shape[0]
    # Folded cosine matrix
    tt = np.arange(1025, dtype=np.float64)
    Cf = np.zeros((TF, SF), np.float32)
    Cf[:1025, :1025] = np.cos(
        2.0 * np.pi * np.outer(tt, tt) / S).astype(np.float32)

    w1p = (w1 * ln_g[:, None]).astype(np.float32)
    b1p = (b1 + ln_b @ w1).astype(np.float32)
    wsum1 = w1p.sum(axis=0, dtype=np.float64).astype(np.float32).reshape(1, D)
    b1pd = (b1p / np.float32(D)).reshape(1, D)
    b2r = np.ascontiguousarray(b2.astype(np.float32).reshape(1, D))

    rev = np.concatenate([[0], np.arange(D - 1, 0, -1)])
    shared = dict(cf=Cf, w1p=w1p, w2=np.ascontiguousarray(w2, dtype=np.float32),
                  wsum1r=wsum1, b1pdr=b1pd, b2r=b2r,
                  onescol=np.ones((128, 1), np.float32),
                  onesrow=np.ones((1, 128), np.float32))

    in_maps = []
    for b in range(B):
        xb = np.asarray(x[b], np.float32)
        w = np.float32(D) * xb[:, rev]
        wf_ = np.zeros((TF, D), np.float32)
        wf_[0] = w[0]
        wf_[1024] = w[1024]
        wf_[1:1024] = w[1:1024] + w[2047:1024:-1]
        xT = np.ascontiguousarray(xb.T)
        in_maps.append(dict(xT=xT, wf=wf_, **shared))
    return in_maps


def _run(inputs, trace=False, trace_kwargs=None):
    x = np.asarray(inputs["x"], np.float32)
    in_maps = _host_prep(
        x,
        np.asarray(inputs["ln_g"], np.float32),
        np.asarray(inputs["ln_b"], np.float32),
        np.asarray(inputs["w1"], np.float32),
        np.asarray(inputs["b1"], np.float32),
        np.asarray(inputs["w2"], np.float32),
        np.asarray(inputs["b2"], np.float32),
    )
    nc = _build_nc()
    res = run_bass_kernel_spmd(nc, in_maps, list(range(NCORES)), trace=trace,
                               **(trace_kwargs or {}))
    outs = np.stack([np.asarray(res.results[b]["out"], np.float32)
                     for b in range(NCORES)])
    outs = outs * np.asarray(inputs["mask"], np.float32)
    return outs, res


def kernel(**inputs) -> np.ndarray:
    out, _ = _run(inputs, trace=False)
    return out



# revision 20
# speedup vs baseline: 1.4826x; 1.4826x over previous
"""FNet transformer block kernel for Trainium2 (8 NeuronCores, data-parallel over batch).

Math notes
----------
reference computes, per batch b:
    ft  = Re( FFT_seq( FFT_hid( FFT_hid( x ))))        (hidden FFT applied twice)
    u   = x + ft;  t = LayerNorm(u) * g + beta
    out = (gelu(t @ w1 + b1) @ w2 + b2) * mask

Double FFT along hidden (D=1024):  (F_D^2 x)[d] = D * x[(-d) mod D]  (real).
So with w[t, d] = 1024 * x[t, (-d) mod 1024]:
    ft = Re(F_S) @ w = C @ w,   C[s, t] = cos(2*pi*s*t/2048)   (S=2048)
C is symmetric in both index reflections, so the 2048x2048 cosine transform
folds to a ~1025x1025 one:  zt = wf.T @ Cf with wf the t-folded w; ft.T
columns for s > 1024 mirror zt columns 2048-s.

Everything downstream stays TRANSPOSED (feature axis on partitions, tokens on
the free axis), with weights as the stationary matmul operand:
    FFN1:  psA[j, s] = sum_d w1p[d, j] * v[d, s] + wsum1[j] * c[s]
           v = u * rbc (token-wise LN scale broadcast),  c = -rsqrt_row * S1
    GELU applies b1p[j] as a free per-partition ACT bias; FFN2 consumes H.T
    directly (no PE transposes), b2[j] rides the PSUM->SBUF ACT copy.
Output is produced as out.T in DRAM; the host transposes it back.
Activations and FFN weights are bf16 (1 cycle/row matmuls, 2x DVE, half SBUF);
the FFT runs in fp32r with N=352 column chunks (full-rate, fits a PSUM bank).
"""

import sys
from contextlib import ExitStack

import numpy as np

sys.path.insert(0, "/opt/trn_rl_repo")

import ml_dtypes  # noqa: E402

import concourse.bass as bass  # noqa: E402
import concourse.mybir as mybir  # noqa: E402
import concourse.tile as tile  # noqa: E402
from concourse import bacc  # noqa: E402
from concourse.bass_utils import run_bass_kernel_spmd  # noqa: E402

S, D = 2048, 1024
TF = 1152  # folded-t rows: 1025 padded up to 9*128
SF = 1056  # folded-s cols: 1025 padded up to 1056
NCORES = 8
LN_EPS = 1e-5
EPS_P = float(D) * float(D) * LN_EPS
F32 = mybir.dt.float32
F32R = mybir.dt.float32r
BF16 = mybir.dt.bfloat16
KT = TF // 128  # 9
DT = D // 128   # 8
SC = 512        # token chunk width
NSC = S // SC   # 4
FCH = [(0, 352), (352, 352), (704, 352)]  # FFT output column chunks of SF
BF = ml_dtypes.bfloat16
AF = mybir.ActivationFunctionType


def _r(ap):
    return ap.bitcast(F32R)


def _emit_kernel(ctx: ExitStack, tc: tile.TileContext, xT, wf, cf, w1b, w2b,
                 wsum1r, b1c, b2c, onesb, onesD, outT):
    nc = tc.nc

    cpool = ctx.enter_context(tc.tile_pool(name="consts", bufs=1))
    ones_col = cpool.tile([128, 1], BF16, tag="ones_col")
    nc.sync.dma_start(ones_col[:], onesb[:])
    onesDi_row = cpool.tile([1, 128], F32R, tag="onesDi_row")
    nc.sync.dma_start(onesDi_row[:], onesD[:])
    eps_t = cpool.tile([1, 1], F32, tag="eps_t")
    nc.gpsimd.memset(eps_t[:], EPS_P)
    wsum1_s = cpool.tile([1, D], BF16, tag="wsum1")
    nc.sync.dma_start(wsum1_s[:], wsum1r[:])
    b1c_s = cpool.tile([128, DT], F32, tag="b1c")
    nc.sync.dma_start(b1c_s[:], b1c[:])
    b2c_s = cpool.tile([128, DT], F32, tag="b2c")
    nc.sync.dma_start(b2c_s[:], b2c[:])

    # FFN weights stay resident (bf16); DMAs issued after the FFT operands
    # so the wf/cf loads win the initial HBM bandwidth race.
    wpool = ctx.enter_context(tc.tile_pool(name="w12", bufs=1))
    w1_s = [wpool.tile([128, D], BF16, tag=f"w1_{dt_}", name=f"w1_{dt_}")
            for dt_ in range(DT)]
    w2_s = [wpool.tile([128, D], BF16, tag=f"w2_{dt_}", name=f"w2_{dt_}")
            for dt_ in range(DT)]

    # zt (folded FFT output), resident through the u-adds
    zpool = ctx.enter_context(tc.tile_pool(name="zt", bufs=1))
    zt_s = [zpool.tile([128, SF], F32, tag=f"zt{m}", name=f"zt{m}")
            for m in range(DT)]

    # ---------------- Phase 1: folded cosine transform ----------------
    with tc.tile_pool(name="fft_in", bufs=1) as fpool, \
         tc.tile_pool(name="fft_ps", bufs=4, space="PSUM") as fps:
        wf_s, cf_s = [], []
        for kt in range(KT):
            a = fpool.tile([128, D], BF16, tag=f"wf{kt}")
            nc.sync.dma_start(a[:], wf[kt * 128:(kt + 1) * 128, :])
            wf_s.append(a)
            c = fpool.tile([128, SF], BF16, tag=f"cf{kt}")
            nc.sync.dma_start(c[:], cf[kt * 128:(kt + 1) * 128, :])
            cf_s.append(c)
        for dt_ in range(DT):
            nc.sync.dma_start(w1_s[dt_][:], w1b[dt_ * 128:(dt_ + 1) * 128, :])
            nc.sync.dma_start(w2_s[dt_][:], w2b[dt_ * 128:(dt_ + 1) * 128, :])
        for mt in range(DT):
            for (n0, nw) in FCH:
                ps = fps.tile([128, 352], F32, tag="fftps")
                for kt in range(KT):
                    nc.tensor.matmul(
                        ps[:, :nw],
                        wf_s[kt][:, mt * 128:(mt + 1) * 128],
                        cf_s[kt][:, n0:n0 + nw],
                        start=(kt == 0), stop=(kt == KT - 1),
                    )
                nc.vector.tensor_copy(zt_s[mt][:, n0:n0 + nw], ps[:, :nw])

    # ---------------- Phase 2: residual + LN + FFN, fully transposed ----
    xpool = ctx.enter_context(tc.tile_pool(name="xs", bufs=4))
    upool = ctx.enter_context(tc.tile_pool(name="u", bufs=1))
    u_s = [upool.tile([128, S], BF16, tag=f"u{d}", name=f"u{d}")
           for d in range(DT)]
    vpool = ctx.enter_context(tc.tile_pool(name="v", bufs=1))
    v_s = [vpool.tile([128, S], BF16, tag=f"v{d}", name=f"v{d}")
           for d in range(DT)]
    hpool = ctx.enter_context(tc.tile_pool(name="h", bufs=1))
    h_s = [hpool.tile([128, S], BF16, tag=f"h{j}", name=f"h{j}")
           for j in range(DT)]
    usqpool = ctx.enter_context(tc.tile_pool(name="usq", bufs=8))
    rowpool = ctx.enter_context(tc.tile_pool(name="rows", bufs=1))
    rbpool = ctx.enter_context(tc.tile_pool(name="rb", bufs=2))
    opool = ctx.enter_context(tc.tile_pool(name="o", bufs=3))
    s1ps = ctx.enter_context(tc.tile_pool(name="s1ps", bufs=1, space="PSUM"))
    s2ps = ctx.enter_context(tc.tile_pool(name="s2ps", bufs=1, space="PSUM"))
    rbps = ctx.enter_context(tc.tile_pool(name="rbps", bufs=1, space="PSUM"))
    aps = ctx.enter_context(tc.tile_pool(name="aps", bufs=2, space="PSUM"))
    ops_ = ctx.enter_context(tc.tile_pool(name="ops", bufs=2, space="PSUM"))

    def emit_prep(sc):
        """DVE: u = x + mirrored zt (bf16 out); usq = u*u (bf16)."""
        s0 = sc * SC
        for d in range(DT):
            xt = xpool.tile([128, SC], F32, tag="xt")
            nc.gpsimd.dma_start(xt[:], xT[d * 128:(d + 1) * 128, s0:s0 + SC])
            z = zt_s[d]
            u = u_s[d]
            if sc <= 1:
                nc.vector.tensor_add(u[:, s0:s0 + SC], xt[:], z[:, s0:s0 + SC])
            elif sc == 2:
                nc.vector.tensor_add(u[:, 1024:1025], xt[:, 0:1],
                                     z[:, 1024:1025])
                nc.vector.tensor_add(u[:, 1025:1536], xt[:, 1:SC],
                                     z[:, 1023:512:-1])
            else:
                nc.vector.tensor_add(u[:, 1536:2048], xt[:], z[:, 512:0:-1])
        usq = []
        for d in range(DT):
            q = usqpool.tile([128, SC], BF16, tag="usq")
            nc.vector.tensor_mul(q[:], u_s[d][:, s0:s0 + SC],
                                 u_s[d][:, s0:s0 + SC])
            usq.append(q)
        return usq

    def emit_stats(sc, usq):
        """PE token stats + LN rows + rbc broadcast + v tiles."""
        s0 = sc * SC
        s1 = s1ps.tile([1, SC], F32, tag="s1")
        for d in range(DT):
            nc.tensor.matmul(s1[0:1, :], ones_col[:], u_s[d][:, s0:s0 + SC],
                             start=(d == 0), stop=(d == DT - 1))
        s2 = s2ps.tile([1, SC], F32, tag="s2")
        for d in range(DT):
            nc.tensor.matmul(s2[0:1, :], ones_col[:], usq[d][:],
                             start=(d == 0), stop=(d == DT - 1))
        s1r = rowpool.tile([1, SC], F32, tag="s1r")
        nc.vector.tensor_copy(s1r[:], s1[0:1, :])
        s2r = rowpool.tile([1, SC], F32, tag="s2r")
        nc.vector.tensor_copy(s2r[:], s2[0:1, :])
        sq = rowpool.tile([1, SC], F32, tag="sq")
        nc.vector.tensor_mul(sq[:], s1r[:], s1r[:])
        vr = rowpool.tile([1, SC], F32, tag="vr")
        # vr = D*S2 - S1^2  (= D^2 * var)
        nc.vector.scalar_tensor_tensor(
            out=vr[:], in0=s2r[:], scalar=float(D), in1=sq[:],
            op0=mybir.AluOpType.mult, op1=mybir.AluOpType.subtract)
        svr = rowpool.tile([1, SC], F32R, tag="svr")
        nc.scalar.activation(svr[:], vr[:], AF.Sqrt,
                             bias=eps_t[0:1, 0:1], scale=1.0)
        # pbs = (1/D) * ones x svr  ->  full-tile broadcast of svr/D, so the
        # reciprocal runs partition-parallel (a [1,512] row reciprocal is
        # serial on one DVE lane and costs ~4us).
        pbs = rbps.tile([128, SC], F32, tag="pbs")
        nc.tensor.matmul(pbs[:], onesDi_row[:], svr[:], start=True, stop=True)
        rb32 = rbpool.tile([128, SC], F32, tag="rb32")
        with nc.allow_low_precision(reason="LN scale needs ~8 sig bits"):
            nc.vector.reciprocal(rb32[:], pbs[:])  # = D/svr = 1/sqrt(var+eps)
        rb = rbpool.tile([128, SC], BF16, tag="rb")
        nc.vector.tensor_copy(rb[:], rb32[:])
        crow = rowpool.tile([1, SC], BF16, tag="crow", bufs=2)
        # crow = -(S1/D) * r_true = -mu * r
        nc.vector.scalar_tensor_tensor(
            out=crow[:], in0=s1r[:], scalar=-1.0 / float(D),
            in1=rb32[0:1, :],
            op0=mybir.AluOpType.mult, op1=mybir.AluOpType.mult)
        for d in range(DT):
            nc.vector.tensor_mul(v_s[d][:, s0:s0 + SC],
                                 u_s[d][:, s0:s0 + SC], rb[:])
        return crow

    def emit_f1(sc, crow):
        s0 = sc * SC
        for j in range(DT):
            pa = aps.tile([128, SC], F32, tag="pa")
            for d in range(DT):
                nc.tensor.matmul(pa[:], w1_s[d][:, j * 128:(j + 1) * 128],
                                 v_s[d][:, s0:s0 + SC],
                                 start=(d == 0), stop=False)
            nc.tensor.matmul(pa[:], wsum1_s[0:1, j * 128:(j + 1) * 128],
                             crow[:], start=False, stop=True)
            nc.scalar.activation(h_s[j][:, s0:s0 + SC], pa[:], AF.Gelu,
                                 bias=b1c_s[:, j:j + 1], scale=1.0)

    def emit_f2(sc):
        s0 = sc * SC
        for do in range(DT):
            po = ops_.tile([128, SC], F32, tag="po")
            for j in range(DT):
                nc.tensor.matmul(po[:], w2_s[j][:, do * 128:(do + 1) * 128],
                                 h_s[j][:, s0:s0 + SC],
                                 start=(j == 0), stop=(j == DT - 1))
            o = opool.tile([128, SC], F32, tag="o")
            nc.scalar.activation(o[:], po[:], AF.Identity,
                                 bias=b2c_s[:, do:do + 1], scale=1.0)
            nc.gpsimd.dma_start(outT[do * 128:(do + 1) * 128, s0:s0 + SC], o[:])

    # Interleaved schedule: PE always has FFN work queued behind each stats
    # segment so the DVE/ACT LN chain latency is hidden.
    usq0 = emit_prep(0)
    c0 = emit_stats(0, usq0)
    usq1 = emit_prep(1)
    emit_f1(0, c0)
    c1 = emit_stats(1, usq1)
    usq2 = emit_prep(2)
    emit_f1(1, c1)
    c2 = emit_stats(2, usq2)
    emit_f2(0)
    usq3 = emit_prep(3)
    emit_f1(2, c2)
    c3 = emit_stats(3, usq3)
    emit_f2(1)
    emit_f1(3, c3)
    emit_f2(2)
    emit_f2(3)


_NC_CACHE = {}


def _build_nc():
    if "nc" in _NC_CACHE:
        return _NC_CACHE["nc"]
    nc = bacc.Bacc("TRN2", target_bir_lowering=False, debug=False)
    xT = nc.declare_dram_parameter("xT", [D, S], F32, isOutput=False)
    wf = nc.declare_dram_parameter("wf", [TF, D], BF16, isOutput=False)
    cf = nc.declare_dram_parameter("cf", [TF, SF], BF16, isOutput=False)
    w1b = nc.declare_dram_parameter("w1b", [D, D], BF16, isOutput=False)
    w2b = nc.declare_dram_parameter("w2b", [D, D], BF16, isOutput=False)
    wsum1r = nc.declare_dram_parameter("wsum1r", [1, D], BF16, isOutput=False)
    b1c = nc.declare_dram_parameter("b1c", [128, DT], F32, isOutput=False)
    b2c = nc.declare_dram_parameter("b2c", [128, DT], F32, isOutput=False)
    onesb = nc.declare_dram_parameter("onesb", [128, 1], BF16, isOutput=False)
    onesD = nc.declare_dram_parameter("onesD", [1, 128], F32R, isOutput=False)
    outT = nc.declare_dram_parameter("outT", [D, S], F32, isOutput=True)
    with tile.TileContext(nc) as tc:
        with ExitStack() as ctx:
            _emit_kernel(ctx, tc, xT, wf, cf, w1b, w2b, wsum1r, b1c, b2c,
                         onesb, onesD, outT)
    nc.compile()
    _NC_CACHE["nc"] = nc
    return nc


def _host_prep(x, ln_g, ln_b, w1, b1, w2, b2):
    """Build per-core and shared device inputs."""
    B = x.shape[0]
    tt = np.arange(1025, dtype=np.float64)
    Cf = np.zeros((TF, SF), BF)
    Cf[:1025, :1025] = np.cos(
        2.0 * np.pi * np.outer(tt, tt) / S).astype(BF)

    w1p = (w1 * ln_g[:, None]).astype(np.float32)
    w1pb = w1p.astype(BF)
    w2b_ = np.asarray(w2, np.float32).astype(BF)
    wsum1 = w1pb.astype(np.float64).sum(axis=0).astype(BF).reshape(1, D)
    b1p = (b1 + ln_b @ w1).astype(np.float32)
    b1c_ = np.ascontiguousarray(b1p.reshape(DT, 128).T)
    b2c_ = np.ascontiguousarray(np.asarray(b2, np.float32).reshape(DT, 128).T)

    rev = np.concatenate([[0], np.arange(D - 1, 0, -1)])
    shared = dict(cf=Cf, w1b=w1pb, w2b=w2b_, wsum1r=wsum1, b1c=b1c_, b2c=b2c_,
                  onesb=np.ones((128, 1), BF),
                  onesD=np.full((1, 128), 1.0 / float(D), np.float32))

    in_maps = []
    for b in range(B):
        xb = np.asarray(x[b], np.float32)
        w = np.float32(D) * xb[:, rev]
        wf_ = np.zeros((TF, D), BF)
        wf_[0] = w[0]
        wf_[1024] = w[1024]
        wf_[1:1024] = (w[1:1024] + w[2047:1024:-1]).astype(BF)
        xT = np.ascontiguousarray(xb.T)
        in_maps.append(dict(xT=xT, wf=wf_, **shared))
    return in_maps


def _run(inputs, trace=False, trace_kwargs=None):
    x = np.asarray(inputs["x"], np.float32)
    in_maps = _host_prep(
        x,
        np.asarray(inputs["ln_g"], np.float32),
        np.asarray(inputs["ln_b"], np.float32),
        np.asarray(inputs["w1"], np.float32),
        np.asarray(inputs["b1"], np.float32),
        np.asarray(inputs["w2"], np.float32),
        np.asarray(inputs["b2"], np.float32),
    )
    nc = _build_nc()
    res = run_bass_kernel_spmd(nc, in_maps, list(range(NCORES)), trace=trace,
                               **(trace_kwargs or {}))
    outs = np.stack([np.ascontiguousarray(
        np.asarray(res.results[b]["outT"], np.float32).T)
        for b in range(NCORES)])
    outs = outs * np.asarray(inputs["mask"], np.float32)
    return outs, res


def kernel(**inputs) -> np.ndarray:
    out, _ = _run(inputs, trace=False)
    return out
